# revision 21
# baseline (speedup 1.0000x reference)
"""ConvFFN block kernel for 8 Trainium2 NeuronCores.

Problem: x (8,128,64,1024) f32;
  stage 1: per-d (D=128) 64x64 channel-mixing matmuls over m, gelu between;
  transpose (b d m n -> b m d n);
  stage 2: per-m (M=64) 128x128 channel-mixing matmuls over d, gelu between;
  transpose back, residual add.

Sharding: data-parallel over batch B=8, one batch per core, no collectives.

Per-core dataflow (D=128, M=64, N=1024):
  - stage-1 groups are paired: block-diagonal 128x128 weights process two
    d-groups per matmul at full PE-array utilization.
  - L1a: h1 = W1a_bd.T @ x_pair            (bf16, PSUM f32)
  - gelu1 on ACT evicts PSUM -> SBUF bf16 with per-partition bias b1a.
  - L1b is fused with the d<->m transpose: out[n,(d',m)] = g1_pair.T @ W1b_bd
    puts n on partitions, so the W1b multiply rides the layout flip.
  - U tiles [n_sub=128, (m,d)] collect all pairs; a PE transpose per (m,sub)
    gathers d onto partitions: V[d, n] per m.
  - L2a: W2a[m].T stationary over V; gelu2 with bias b2a_eff
    (b1b folded into b2a_eff on host); L2b: W2b[m].T.
  - final DVE scalar_tensor_tensor: out = (psum + b2b[m,:]) + x  (f32 residual,
    x re-read m-major), stored m-major to DRAM.
"""

import sys

for _p in ("/opt/trn_rl_repo", "/opt/pypackages"):
    if _p not in sys.path:
        sys.path.append(_p)

import numpy as np
import ml_dtypes

from concourse import bacc, tile, mybir
from concourse.bass_utils import run_bass_kernel_spmd

BF16 = mybir.dt.bfloat16
F32 = mybir.dt.float32
AF = mybir.ActivationFunctionType
ALU = mybir.AluOpType

B, D, M, N = 8, 128, 64, 1024
PAIRS = D // 2          # 64 block-diagonal pair groups in stage 1
CH = 256                # n-chunk (columns) processed per pipeline pass
NCH = N // CH           # 4 chunks
SUBS = CH // 128        # 128-col sub-chunks per chunk (transpose granularity)
ROWS = D * M            # 8192 flattened channel rows

_CACHE = {}


def _build_module():
    nc = bacc.Bacc("TRN2", target_bir_lowering=False, debug=False, num_devices=8)

    # xb rows: (c, dl, m); cols: (p, t)   -- 8KB contiguous per partition/load
    xb_d = nc.dram_tensor("xb", [NCH * 128, PAIRS * CH], BF16,
                          kind="ExternalInput").ap()
    # xr/out rows: (c, d); cols: (m, t)    -- 4-8KB contiguous per partition
    xr_d = nc.dram_tensor("xr", [NCH * 128, M * CH], F32,
                          kind="ExternalInput").ap()
    w1a_d = nc.dram_tensor("w1a", [128, PAIRS, 128], BF16, kind="ExternalInput").ap()
    w1b_d = nc.dram_tensor("w1b", [128, PAIRS, 128], BF16, kind="ExternalInput").ap()
    w2a_d = nc.dram_tensor("w2a", [128, M, 128], BF16, kind="ExternalInput").ap()
    w2b_d = nc.dram_tensor("w2b", [128, M, 128], BF16, kind="ExternalInput").ap()
    b1a_d = nc.dram_tensor("b1a_t", [128, PAIRS], F32, kind="ExternalInput").ap()
    b2a_d = nc.dram_tensor("b2a_t", [128, M], F32, kind="ExternalInput").ap()
    b2b_d = nc.dram_tensor("b2b_t", [128, M], F32, kind="ExternalInput").ap()
    id_d = nc.dram_tensor("ident", [128, 128], BF16, kind="ExternalInput").ap()
    out_d = nc.dram_tensor("out", [NCH * 128, M * CH], F32,
                           kind="ExternalOutput").ap()

    MGRP = 8   # m-groups per xr/out DMA batch
    XGRP = 16  # pairs per xb load tile

    with tile.TileContext(nc) as tc:
        with (
            tc.tile_pool(name="wpool", bufs=1) as wpool,
            tc.tile_pool(name="xbp", bufs=2) as xbp,
            tc.tile_pool(name="g1p", bufs=6) as g1p,
            tc.tile_pool(name="up", bufs=4) as up,
            tc.tile_pool(name="vp", bufs=6) as vp,
            tc.tile_pool(name="g2p", bufs=6) as g2p,
            tc.tile_pool(name="xrp", bufs=2) as xrp,
            tc.tile_pool(name="outp", bufs=2) as outp,
            tc.tile_pool(name="ps_a", bufs=2, space="PSUM") as ps_a,
            tc.tile_pool(name="ps_b", bufs=3, space="PSUM") as ps_b,
            tc.tile_pool(name="ps_v", bufs=1, space="PSUM") as ps_v,
            tc.tile_pool(name="ps_2", bufs=2, space="PSUM") as ps_2,
        ):
            # resident weights / constants
            w1a_s = wpool.tile([128, PAIRS * 128], BF16)
            w1b_s = wpool.tile([128, PAIRS * 128], BF16)
            w2a_s = wpool.tile([128, M * 128], BF16)
            w2b_s = wpool.tile([128, M * 128], BF16)
            b1a_s = wpool.tile([128, PAIRS], F32)
            b2a_s = wpool.tile([128, M], F32)
            b2b_s = wpool.tile([128, M], F32)
            id_s = wpool.tile([128, 128], BF16)

            nc.sync.dma_start(
                out=w1a_s[:].rearrange("k (p j) -> k p j", j=128), in_=w1a_d[:]
            )
            nc.sync.dma_start(out=b1a_s[:], in_=b1a_d[:])
            for t, d in ((w1b_s, w1b_d), (w2a_s, w2a_d), (w2b_s, w2b_d)):
                nc.sync.dma_start(
                    out=t[:].rearrange("k (p j) -> k p j", j=128), in_=d[:]
                )
            for t, d in ((b2a_s, b2a_d), (b2b_s, b2b_d), (id_s, id_d)):
                nc.sync.dma_start(out=t[:], in_=d[:])

            for c in range(NCH):
                # ---- stage 1 + fused transpose-matmul, per pair
                u_ts = [up.tile([128, ROWS], BF16, tag="u", name=f"u{c}_{s}")
                        for s in range(SUBS)]
                pb_ts = {}
                xb_t = None
                for p in range(PAIRS):
                    if p % XGRP == 0:
                        xb_t = xbp.tile([128, XGRP * CH], BF16, tag="xb",
                                        name=f"xb{c}_{p}")
                        nc.sync.dma_start(
                            out=xb_t[:],
                            in_=xb_d[c * 128:(c + 1) * 128,
                                     p * CH:(p + XGRP) * CH],
                        )
                    pa = ps_a.tile([128, CH], F32, tag="pa")
                    nc.tensor.matmul(
                        pa[:],
                        w1a_s[:, p * 128:(p + 1) * 128],
                        xb_t[:, (p % XGRP) * CH:(p % XGRP + 1) * CH],
                        start=True, stop=True,
                    )
                    g1 = g1p.tile([128, CH], BF16, tag="g1")
                    nc.scalar.activation(
                        g1[:], pa[:], AF.Gelu, bias=b1a_s[:, p:p + 1], scale=1.0
                    )
                    for s in range(SUBS):
                        if (s, p // 4) not in pb_ts:
                            pb_ts[(s, p // 4)] = ps_b.tile(
                                [128, 512], F32, tag="pb",
                                name=f"pb{c}_{s}_{p // 4}")
                        pb = pb_ts[(s, p // 4)]
                        nc.tensor.matmul(
                            pb[:, (p % 4) * 128:(p % 4 + 1) * 128],
                            g1[:, s * 128:(s + 1) * 128],
                            w1b_s[:, p * 128:(p + 1) * 128],
                            start=True, stop=True,
                        )
                    if p % 4 == 3:
                        # evict 4 pairs into U[n_sub, m*128 + d], d = 2p+dl
                        for s in range(SUBS):
                            pb = pb_ts.pop((s, p // 4))
                            base = (p - 3) * 2
                            dst = (
                                u_ts[s][:]
                                .rearrange("n (m pp) -> n m pp", pp=128)
                                [:, :, base:base + 8]
                                .rearrange("n m (q dl) -> n m q dl", dl=2)
                            )
                            src_ = pb[:].rearrange(
                                "n (q dl m) -> n m q dl", q=4, dl=2
                            )
                            nc.vector.tensor_copy(dst, src_)

                # ---- stage 2 per m: transpose-gather, L2a, gelu2, L2b, STT
                xr_t = None
                out_t = None
                for m in range(M):
                    if m % MGRP == 0:
                        xr_t = xrp.tile([128, MGRP * CH], F32, tag="xr")
                        nc.sync.dma_start(
                            out=xr_t[:],
                            in_=xr_d[c * 128:(c + 1) * 128,
                                     m * CH:(m + MGRP) * CH],
                        )
                        out_t = outp.tile([128, MGRP * CH], F32, tag="out")
                    pv = ps_v.tile([128, CH], BF16, tag="pv")
                    for s in range(SUBS):
                        nc.tensor.transpose(
                            pv[:, s * 128:(s + 1) * 128],
                            u_ts[s][:, m * 128:(m + 1) * 128],
                            id_s[:],
                        )
                    v_t = vp.tile([128, CH], BF16, tag="v")
                    nc.vector.tensor_copy(v_t[:], pv[:])
                    p2 = ps_2.tile([128, CH], F32, tag="p2", name=f"p2{c}_{m}")
                    nc.tensor.matmul(
                        p2[:], w2a_s[:, m * 128:(m + 1) * 128], v_t[:],
                        start=True, stop=True,
                    )
                    g2 = g2p.tile([128, CH], BF16, tag="g2")
                    nc.scalar.activation(
                        g2[:], p2[:], AF.Gelu, bias=b2a_s[:, m:m + 1], scale=1.0
                    )
                    p3 = ps_2.tile([128, CH], F32, tag="p2", name=f"p3{c}_{m}")
                    nc.tensor.matmul(
                        p3[:], w2b_s[:, m * 128:(m + 1) * 128], g2[:],
                        start=True, stop=True,
                    )
                    mi = m % MGRP
                    nc.vector.scalar_tensor_tensor(
                        out_t[:, mi * CH:(mi + 1) * CH],
                        p3[:],
                        b2b_s[:, m:m + 1],
                        xr_t[:, mi * CH:(mi + 1) * CH],
                        ALU.add,
                        ALU.add,
                    )
                    if m % MGRP == MGRP - 1:
                        nc.sync.dma_start(
                            out=out_d[c * 128:(c + 1) * 128,
                                      (m - MGRP + 1) * CH:(m + 1) * CH],
                            in_=out_t[:],
                        )

    nc.compile()
    return nc


def _host_prep(x, W1a, b1a, W1b, b1b, W2a, b2a, W2b, b2b):
    bf16 = ml_dtypes.bfloat16
    x = np.ascontiguousarray(x, dtype=np.float32)
    # xb layout: rows (c, dl, m), cols (p, t);  d = 2p + dl
    xb = np.ascontiguousarray(
        x.astype(bf16)
        .reshape(B, PAIRS, 2, M, NCH, CH)
        .transpose(0, 4, 2, 3, 1, 5)
    ).reshape(B, NCH * 128, PAIRS * CH)
    # xr layout: rows (c, d), cols (m, t)
    xr = np.ascontiguousarray(
        x.reshape(B, D, M, NCH, CH).transpose(0, 3, 1, 2, 4)
    ).reshape(B, NCH * 128, M * CH)

    def bd(W):  # (D,64,64) -> block-diag pairs, partition-major (128,64,128)
        A = np.zeros((PAIRS, 128, 128), np.float32)
        A[:, :64, :64] = W[0::2].transpose(0, 2, 1)
        A[:, 64:, 64:] = W[1::2].transpose(0, 2, 1)
        return np.ascontiguousarray(A.transpose(1, 0, 2)).astype(bf16)

    w1a = bd(W1a)
    w1b = bd(W1b)
    w2a = np.ascontiguousarray(
        W2a.transpose(0, 2, 1).transpose(1, 0, 2)).astype(bf16)
    w2b = np.ascontiguousarray(
        W2b.transpose(0, 2, 1).transpose(1, 0, 2)).astype(bf16)

    b1a_t = np.ascontiguousarray(
        b1a.reshape(PAIRS, 2, M).transpose(1, 2, 0).reshape(128, PAIRS)
    ).astype(np.float32)
    b2a_eff = b2a + np.einsum("mod,dm->mo", W2a, b1b)
    b2a_t = np.ascontiguousarray(b2a_eff.T).astype(np.float32)
    b2b_t = np.ascontiguousarray(b2b.T).astype(np.float32)
    ident = np.eye(128, dtype=bf16)

    shared = {
        "w1a": w1a, "w1b": w1b, "w2a": w2a, "w2b": w2b,
        "b1a_t": b1a_t, "b2a_t": b2a_t, "b2b_t": b2b_t, "ident": ident,
    }
    in_maps = [
        {"xb": np.ascontiguousarray(xb[b]),
         "xr": np.ascontiguousarray(xr[b]), **shared}
        for b in range(B)
    ]
    return in_maps


def kernel(x, W1a, b1a, W1b, b1b, W2a, b2a, W2b, b2b, _trace=False, _tmpdir=None):
    if "nc" not in _CACHE:
        _CACHE["nc"] = _build_module()
    nc = _CACHE["nc"]
    in_maps = _host_prep(x, W1a, b1a, W1b, b1b, W2a, b2a, W2b, b2b)
    res = run_bass_kernel_spmd(
        nc, in_maps, list(range(8)), trace=_trace, tmpdir=_tmpdir
    )
    _CACHE["last_result"] = res
    out = np.stack([np.asarray(res.results[b]["out"]) for b in range(B)])
    # undo device layout: rows (c, d), cols (m, t) -> (b, d, m, n)
    out = out.reshape(B, NCH, D, M, CH).transpose(0, 2, 3, 1, 4)
    return np.ascontiguousarray(out).reshape(B, D, M, N).astype(np.float32)


# revision 22
# speedup vs baseline: 1.0173x; 1.0173x over previous
"""ConvFFN block kernel for 8 Trainium2 NeuronCores.

Problem: x (8,128,64,1024) f32;
  stage 1: per-d (D=128) 64x64 channel-mixing matmuls over m, gelu between;
  transpose (b d m n -> b m d n);
  stage 2: per-m (M=64) 128x128 channel-mixing matmuls over d, gelu between;
  transpose back, residual add.

Sharding: data-parallel over batch B=8, one batch per core, no collectives.

Per-core dataflow (D=128, M=64, N=1024):
  - stage-1 groups are paired: block-diagonal 128x128 weights process two
    d-groups per matmul at full PE-array utilization.
  - L1a: h1 = W1a_bd.T @ x_pair            (bf16, PSUM f32)
  - gelu1 on ACT evicts PSUM -> SBUF bf16 with per-partition bias b1a.
  - L1b is fused with the d<->m transpose: out[n,(d',m)] = g1_pair.T @ W1b_bd
    puts n on partitions, so the W1b multiply rides the layout flip.
  - U tiles [n_sub=128, (m,d)] collect all pairs; a PE transpose per (m,sub)
    gathers d onto partitions: V[d, n] per m.
  - L2a: W2a[m].T stationary over V; gelu2 with bias b2a_eff
    (b1b folded into b2a_eff on host); L2b: W2b[m].T.
  - final DVE scalar_tensor_tensor: out = (psum + b2b[m,:]) + x  (f32 residual,
    x re-read m-major), stored m-major to DRAM.
"""

import sys

for _p in ("/opt/trn_rl_repo", "/opt/pypackages"):
    if _p not in sys.path:
        sys.path.append(_p)

import numpy as np
import ml_dtypes

from concourse import bacc, tile, mybir
from concourse.bass_utils import run_bass_kernel_spmd

BF16 = mybir.dt.bfloat16
F32 = mybir.dt.float32
AF = mybir.ActivationFunctionType
ALU = mybir.AluOpType

B, D, M, N = 8, 128, 64, 1024
PAIRS = D // 2          # 64 block-diagonal pair groups in stage 1
CH = 256                # n-chunk (columns) processed per pipeline pass
NCH = N // CH           # 4 chunks
SUBS = CH // 128        # 128-col sub-chunks per chunk (transpose granularity)
ROWS = D * M            # 8192 flattened channel rows

_CACHE = {}


def _build_module():
    nc = bacc.Bacc("TRN2", target_bir_lowering=False, debug=False, num_devices=8)

    # xb rows: (c, dl, m); cols: (p, t)   -- 8KB contiguous per partition/load
    xb_d = nc.dram_tensor("xb", [NCH * 128, PAIRS * CH], BF16,
                          kind="ExternalInput").ap()
    # xr/out rows: (c, d); cols: (m, t)    -- 4-8KB contiguous per partition
    xr_d = nc.dram_tensor("xr", [NCH * 128, M * CH], F32,
                          kind="ExternalInput").ap()
    w1a_d = nc.dram_tensor("w1a", [128, PAIRS, 128], BF16, kind="ExternalInput").ap()
    w1b_d = nc.dram_tensor("w1b", [128, PAIRS, 128], BF16, kind="ExternalInput").ap()
    w2a_d = nc.dram_tensor("w2a", [128, M, 128], BF16, kind="ExternalInput").ap()
    w2b_d = nc.dram_tensor("w2b", [128, M, 128], BF16, kind="ExternalInput").ap()
    b1a_d = nc.dram_tensor("b1a_t", [128, PAIRS], F32, kind="ExternalInput").ap()
    b2a_d = nc.dram_tensor("b2a_t", [128, M], F32, kind="ExternalInput").ap()
    b2b_d = nc.dram_tensor("b2b_t", [128, M], F32, kind="ExternalInput").ap()
    id_d = nc.dram_tensor("ident", [128, 128], BF16, kind="ExternalInput").ap()
    out_d = nc.dram_tensor("out", [NCH * 128, M * CH], F32,
                           kind="ExternalOutput").ap()

    MGRP = 8   # m-groups per xr/out DMA batch
    XGRP = 16  # pairs per xb load tile

    with tile.TileContext(nc) as tc:
        with (
            tc.tile_pool(name="wpool", bufs=1) as wpool,
            tc.tile_pool(name="xbp", bufs=2) as xbp,
            tc.tile_pool(name="g1p", bufs=6) as g1p,
            tc.tile_pool(name="up", bufs=4) as up,
            tc.tile_pool(name="vp", bufs=6) as vp,
            tc.tile_pool(name="g2p", bufs=6) as g2p,
            tc.tile_pool(name="xrp", bufs=2) as xrp,
            tc.tile_pool(name="outp", bufs=2) as outp,
            tc.tile_pool(name="ps_a", bufs=2, space="PSUM") as ps_a,
            tc.tile_pool(name="ps_b", bufs=2, space="PSUM") as ps_b,
            tc.tile_pool(name="ps_v", bufs=2, space="PSUM") as ps_v,
            tc.tile_pool(name="ps_2", bufs=2, space="PSUM") as ps_2,
        ):
            # resident weights / constants
            w1a_s = wpool.tile([128, PAIRS * 128], BF16)
            w1b_s = wpool.tile([128, PAIRS * 128], BF16)
            w2a_s = wpool.tile([128, M * 128], BF16)
            w2b_s = wpool.tile([128, M * 128], BF16)
            b1a_s = wpool.tile([128, PAIRS], F32)
            b2a_s = wpool.tile([128, M], F32)
            b2b_s = wpool.tile([128, M], F32)
            id_s = wpool.tile([128, 128], BF16)

            nc.sync.dma_start(
                out=w1a_s[:].rearrange("k (p j) -> k p j", j=128), in_=w1a_d[:]
            )
            nc.sync.dma_start(out=b1a_s[:], in_=b1a_d[:])
            for t, d in ((w1b_s, w1b_d), (w2a_s, w2a_d), (w2b_s, w2b_d)):
                nc.sync.dma_start(
                    out=t[:].rearrange("k (p j) -> k p j", j=128), in_=d[:]
                )
            for t, d in ((b2a_s, b2a_d), (b2b_s, b2b_d), (id_s, id_d)):
                nc.sync.dma_start(out=t[:], in_=d[:])

            for c in range(NCH):
                # ---- stage 1 + fused transpose-matmul, per pair
                u_ts = [up.tile([128, ROWS], BF16, tag="u", name=f"u{c}_{s}")
                        for s in range(SUBS)]
                pb_ts = {}
                xb_t = None
                for p in range(PAIRS):
                    if p % XGRP == 0:
                        xb_t = xbp.tile([128, XGRP * CH], BF16, tag="xb",
                                        name=f"xb{c}_{p}")
                        nc.sync.dma_start(
                            out=xb_t[:],
                            in_=xb_d[c * 128:(c + 1) * 128,
                                     p * CH:(p + XGRP) * CH],
                        )
                    pa = ps_a.tile([128, CH], F32, tag="pa")
                    nc.tensor.matmul(
                        pa[:],
                        w1a_s[:, p * 128:(p + 1) * 128],
                        xb_t[:, (p % XGRP) * CH:(p % XGRP + 1) * CH],
                        start=True, stop=True,
                    )
                    g1 = g1p.tile([128, CH], BF16, tag="g1")
                    nc.scalar.activation(
                        g1[:], pa[:], AF.Gelu, bias=b1a_s[:, p:p + 1], scale=1.0
                    )
                    for s in range(SUBS):
                        if (s, p // 4) not in pb_ts:
                            pb_ts[(s, p // 4)] = ps_b.tile(
                                [128, 512], F32, tag="pb",
                                name=f"pb{c}_{s}_{p // 4}")
                        pb = pb_ts[(s, p // 4)]
                        nc.tensor.matmul(
                            pb[:, (p % 4) * 128:(p % 4 + 1) * 128],
                            g1[:, s * 128:(s + 1) * 128],
                            w1b_s[:, p * 128:(p + 1) * 128],
                            start=True, stop=True,
                        )
                    if p % 4 == 3:
                        # evict 4 pairs into U[n_sub, m*128 + d], d = 2p+dl
                        for s in range(SUBS):
                            pb = pb_ts.pop((s, p // 4))
                            base = (p - 3) * 2
                            dst = (
                                u_ts[s][:]
                                .rearrange("n (m pp) -> n m pp", pp=128)
                                [:, :, base:base + 8]
                                .rearrange("n m (q dl) -> n m q dl", dl=2)
                            )
                            src_ = pb[:].rearrange(
                                "n (q dl m) -> n m q dl", q=4, dl=2
                            )
                            nc.vector.tensor_copy(dst, src_)

                # ---- stage 2 per m: transpose-gather, L2a, gelu2, L2b, STT
                xr_t = None
                out_t = None
                for m in range(M):
                    if m % MGRP == 0:
                        xr_t = xrp.tile([128, MGRP * CH], F32, tag="xr")
                        nc.sync.dma_start(
                            out=xr_t[:],
                            in_=xr_d[c * 128:(c + 1) * 128,
                                     m * CH:(m + MGRP) * CH],
                        )
                        out_t = outp.tile([128, MGRP * CH], F32, tag="out")
                    pv = ps_v.tile([128, CH], BF16, tag="pv")
                    for s in range(SUBS):
                        nc.tensor.transpose(
                            pv[:, s * 128:(s + 1) * 128],
                            u_ts[s][:, m * 128:(m + 1) * 128],
                            id_s[:],
                        )
                    v_t = vp.tile([128, CH], BF16, tag="v")
                    nc.vector.tensor_copy(v_t[:], pv[:])
                    p2 = ps_2.tile([128, CH], F32, tag="p2", name=f"p2{c}_{m}")
                    nc.tensor.matmul(
                        p2[:], w2a_s[:, m * 128:(m + 1) * 128], v_t[:],
                        start=True, stop=True,
                    )
                    g2 = g2p.tile([128, CH], BF16, tag="g2")
                    nc.scalar.activation(
                        g2[:], p2[:], AF.Gelu, bias=b2a_s[:, m:m + 1], scale=1.0
                    )
                    p3 = ps_2.tile([128, CH], F32, tag="p2", name=f"p3{c}_{m}")
                    nc.tensor.matmul(
                        p3[:], w2b_s[:, m * 128:(m + 1) * 128], g2[:],
                        start=True, stop=True,
                    )
                    mi = m % MGRP
                    nc.vector.scalar_tensor_tensor(
                        out_t[:, mi * CH:(mi + 1) * CH],
                        p3[:],
                        b2b_s[:, m:m + 1],
                        xr_t[:, mi * CH:(mi + 1) * CH],
                        ALU.add,
                        ALU.add,
                    )
                    if m % MGRP == MGRP - 1:
                        nc.sync.dma_start(
                            out=out_d[c * 128:(c + 1) * 128,
                                      (m - MGRP + 1) * CH:(m + 1) * CH],
                            in_=out_t[:],
                        )

    nc.compile()
    return nc


def _host_prep(x, W1a, b1a, W1b, b1b, W2a, b2a, W2b, b2b):
    bf16 = ml_dtypes.bfloat16
    x = np.ascontiguousarray(x, dtype=np.float32)
    # xb layout: rows (c, dl, m), cols (p, t);  d = 2p + dl
    xb = np.ascontiguousarray(
        x.astype(bf16)
        .reshape(B, PAIRS, 2, M, NCH, CH)
        .transpose(0, 4, 2, 3, 1, 5)
    ).reshape(B, NCH * 128, PAIRS * CH)
    # xr layout: rows (c, d), cols (m, t)
    xr = np.ascontiguousarray(
        x.reshape(B, D, M, NCH, CH).transpose(0, 3, 1, 2, 4)
    ).reshape(B, NCH * 128, M * CH)

    def bd(W):  # (D,64,64) -> block-diag pairs, partition-major (128,64,128)
        A = np.zeros((PAIRS, 128, 128), np.float32)
        A[:, :64, :64] = W[0::2].transpose(0, 2, 1)
        A[:, 64:, 64:] = W[1::2].transpose(0, 2, 1)
        return np.ascontiguousarray(A.transpose(1, 0, 2)).astype(bf16)

    w1a = bd(W1a)
    w1b = bd(W1b)
    w2a = np.ascontiguousarray(
        W2a.transpose(0, 2, 1).transpose(1, 0, 2)).astype(bf16)
    w2b = np.ascontiguousarray(
        W2b.transpose(0, 2, 1).transpose(1, 0, 2)).astype(bf16)

    b1a_t = np.ascontiguousarray(
        b1a.reshape(PAIRS, 2, M).transpose(1, 2, 0).reshape(128, PAIRS)
    ).astype(np.float32)
    b2a_eff = b2a + np.einsum("mod,dm->mo", W2a, b1b)
    b2a_t = np.ascontiguousarray(b2a_eff.T).astype(np.float32)
    b2b_t = np.ascontiguousarray(b2b.T).astype(np.float32)
    ident = np.eye(128, dtype=bf16)

    shared = {
        "w1a": w1a, "w1b": w1b, "w2a": w2a, "w2b": w2b,
        "b1a_t": b1a_t, "b2a_t": b2a_t, "b2b_t": b2b_t, "ident": ident,
    }
    in_maps = [
        {"xb": np.ascontiguousarray(xb[b]),
         "xr": np.ascontiguousarray(xr[b]), **shared}
        for b in range(B)
    ]
    return in_maps


def kernel(x, W1a, b1a, W1b, b1b, W2a, b2a, W2b, b2b, _trace=False, _tmpdir=None):
    if "nc" not in _CACHE:
        _CACHE["nc"] = _build_module()
    nc = _CACHE["nc"]
    in_maps = _host_prep(x, W1a, b1a, W1b, b1b, W2a, b2a, W2b, b2b)
    res = run_bass_kernel_spmd(
        nc, in_maps, list(range(8)), trace=_trace, tmpdir=_tmpdir
    )
    _CACHE["last_result"] = res
    out = np.stack([np.asarray(res.results[b]["out"]) for b in range(B)])
    # undo device layout: rows (c, d), cols (m, t) -> (b, d, m, n)
    out = out.reshape(B, NCH, D, M, CH).transpose(0, 2, 3, 1, 4)
    return np.ascontiguousarray(out).reshape(B, D, M, N).astype(np.float32)
